# revision 18
# baseline (speedup 1.0000x reference)
"""Trainium2 Bass kernel for CIF (Continuous Integrate-and-Fire), batch-parallel over 8 cores.

Per core (4 samples):
  - small pipeline: clip/mask weights, two-level f32 cumsum -> h
  - per 128-step chunk j: build banded operator tile
        A[s, t] = clip(min(h_s - t, (t+1) - hprev_s), 0, w_s)
    over a STATIC 256-wide 128-aligned t-window W_j (cumsum deviation from its
    linear trend is a Brownian bridge, sigma ~5, window margin ~30 = 6 sigma;
    verified on host with a fully-static 512-wide fallback program), then
    contract with x on the PE into statically-routed PSUM banks.
  - delay[t] rides as a transposed matmul (lhsT = src column) into a [1,512] row
  - dur via sigma-h occupancy counts: ones-matmul over comparison tiles + a
    constant staircase for chunks fully below each grid point
  - w_sum / right_w / left_w from the small per-s pipeline
"""

import os
import sys

import numpy as np

for _p in ("/opt/trn_rl_repo", "/root/.axon_site/_ro/trn_rl_repo"):
    if os.path.isdir(_p) and _p not in sys.path:
        sys.path.insert(0, _p)

import concourse.bass as bass
import concourse.bacc as bacc
import concourse.tile as tile
from concourse import mybir
from concourse.bass_utils import run_bass_kernel_spmd

F32 = mybir.dt.float32
I32 = mybir.dt.int32
U8 = mybir.dt.uint8
OP = mybir.AluOpType
ACTF = mybir.ActivationFunctionType

B, S, C, TMAX = 32, 2048, 512, 512
NCORES = 8
BLOC = B // NCORES
NCH, CH = 16, 128
MAGIC = float(np.float32(1.5 * 2.0**23))

# static t-window base per chunk (128-aligned, 256 wide) and sigma-h grid window
WBASE = [128 * min(max((32 * c - 64) >> 7, 0), 2) for c in range(NCH)]
GBASE = [128 * min(max((32 * c - 64) >> 7, 0), 3) for c in range(NCH)]

CONFIGS = {
    "banded": {"wbase": WBASE, "wwidth": 256, "gbase": GBASE, "gwidth": 256},
    "full": {"wbase": [0] * NCH, "wwidth": 512, "gbase": [0] * NCH, "gwidth": 640},
}


def _emit_consts(nc, tc, pools, cfg):
    pc = pools["const"]
    cst = {}
    # master index row [128, 772] f32: value = free index (same on all partitions)
    ti = pc.tile([128, 772], I32, tag="ti")
    nc.gpsimd.iota(ti[:], [[1, 772]], base=0, channel_multiplier=0)
    trow = pc.tile([128, 772], F32, tag="trow")
    nc.vector.tensor_copy(trow[:], ti[:])
    cst["trow"] = trow
    # partition-index column [128,1] f32
    pi = pc.tile([128, 1], I32, tag="pi")
    nc.gpsimd.iota(pi[:], [[1, 1]], base=0, channel_multiplier=1)
    pcol = pc.tile([128, 1], F32, tag="pcol")
    nc.vector.tensor_copy(pcol[:], pi[:])
    cst["pcol"] = pcol
    # identity [128,128] and strict-upper-tri [16,16]
    ir = pc.tile([128, 128], I32, tag="ir")
    nc.gpsimd.iota(ir[:], [[1, 128]], base=0, channel_multiplier=0)
    ic = pc.tile([128, 128], I32, tag="ic")
    nc.gpsimd.iota(ic[:], [[0, 128]], base=0, channel_multiplier=1)
    ident = pc.tile([128, 128], F32, tag="ident")
    nc.vector.tensor_tensor(ident[:], ir[:], ic[:], OP.is_equal)
    cst["ident"] = ident
    triu = pc.tile([16, 16], F32, tag="triu")
    nc.vector.tensor_tensor(triu[:], ic[0:16, 0:16], ir[0:16, 0:16], OP.is_lt)
    cst["triu"] = triu
    ones16 = pc.tile([16, 1], F32, tag="ones16")
    nc.vector.memset(ones16[:], 1.0)
    cst["ones16"] = ones16
    ones_1x16 = pc.tile([1, 16], F32, tag="ones_1x16")
    nc.vector.memset(ones_1x16[:], 1.0)
    cst["ones_1x16"] = ones_1x16
    ones128 = pc.tile([128, 1], F32, tag="ones128")
    nc.vector.memset(ones128[:], 1.0)
    cst["ones128"] = ones128
    zeros128 = pc.tile([128, 512], F32, tag="zeros128")
    nc.vector.memset(zeros128[:], 0.0)
    cst["zeros128"] = zeros128
    z16 = pc.tile([16, 128], F32, tag="z16")
    nc.vector.memset(z16[:], 0.0)
    cst["z16"] = z16
    c128_16 = pc.tile([16, 1], F32, tag="c128_16")
    nc.vector.memset(c128_16[:], 128.0)
    cst["c128_16"] = c128_16
    # sigma-h step comparison tile [16, 768]: step16[c, j] = 1[j >= gbase_c + gwidth]
    # (128 * column-sum = count of chunks entirely below grid point j).
    # thr_c = gbase_c + gwidth with gbase_c = 128*clamp(floor(c/4 - 0.5), 0, 3):
    ccol_i = pc.tile([16, 1], I32, tag="ccol_i")
    nc.gpsimd.iota(ccol_i[:], [[1, 1]], base=0, channel_multiplier=1)
    ccol = pc.tile([16, 1], F32, tag="ccol")
    nc.vector.tensor_copy(ccol[:], ccol_i[:])
    u16 = pc.tile([16, 1], F32, tag="u16")
    nc.vector.tensor_scalar(u16[:], ccol[:], 0.25, -0.99, OP.mult, OP.add)
    f16 = pc.tile([16, 1], F32, tag="f16")
    nc.vector.tensor_scalar(f16[:], u16[:], MAGIC, MAGIC, OP.add, OP.subtract)
    g16 = pc.tile([16, 1], F32, tag="g16")
    nc.vector.tensor_scalar(g16[:], f16[:], 0.0, 3.0, OP.max, OP.min)
    thr16 = pc.tile([16, 1], F32, tag="thr16")
    nc.vector.tensor_scalar(thr16[:], g16[:], 128.0, float(cfg["gwidth"]), OP.mult, OP.add)
    if cfg["gwidth"] >= 640:
        # full config: window covers the whole real grid; no step term
        nc.vector.tensor_scalar(thr16[:], thr16[:], 0.0, 100000.0, OP.mult, OP.add)
    step16 = pc.tile([16, 768], F32, tag="step16")
    nc.vector.tensor_scalar(step16[:], trow[0:16, 0:768], thr16[:], None, OP.is_ge)
    cst["step16"] = step16
    return cst


def _rne(V, out, in_):
    V.tensor_scalar(out, in_, MAGIC, MAGIC, OP.add, OP.subtract)


def _emit_sample(nc, tc, pools, cst, io, b, cfg):
    sm = pools["sm"]
    ps_sm = pools["ps_sm"]
    V, G, A_, P = nc.vector, nc.gpsimd, nc.scalar, nc.tensor
    wbase, wwidth = cfg["wbase"], cfg["wwidth"]
    gbase, gwidth = cfg["gbase"], cfg["gwidth"]

    # ---- load small inputs ----
    iw = sm.tile([16, 128], F32, tag="iw")
    nc.sync.dma_start(iw[:], io["iw"][b].rearrange("(n c) -> n c", n=16))
    pmt = sm.tile([16, 128], U8, tag="pmt")
    nc.sync.dma_start(pmt[:], io["pm"][b].rearrange("(n c) -> n c", n=16))
    tli = sm.tile([1, 1], I32, tag="tli")
    nc.sync.dma_start(tli[:], io["tl"][b : b + 1, :])

    # ---- w = clip(iw,0,1) * (1-mask) ----
    pm_f = sm.tile([16, 128], F32, tag="pm_f")
    G.tensor_copy(pm_f[:], pmt[:])
    notm = sm.tile([16, 128], F32, tag="notm")
    G.tensor_scalar(notm[:], pm_f[:], -1.0, 1.0, OP.mult, OP.add)
    w0 = sm.tile([16, 128], F32, tag="w0")
    G.tensor_scalar(w0[:], iw[:], 0.0, 1.0, OP.max, OP.min)
    w = sm.tile([16, 128], F32, tag="w")
    V.tensor_tensor(w[:], w0[:], notm[:], OP.mult)

    # ---- w_sum ----
    csum_col = sm.tile([16, 1], F32, tag="csum_col")
    V.tensor_reduce(csum_col[:], w[:], mybir.AxisListType.X, OP.add)
    ps1 = ps_sm.tile([128, 128], F32, tag="pssm")
    P.matmul(ps1[0:1, 0:1], csum_col[:], cst["ones16"][:], start=True, stop=True)
    wsum = sm.tile([1, 1], F32, tag="wsum")
    V.tensor_copy(wsum[:], ps1[0:1, 0:1])
    nc.sync.dma_start(io["ws"][b : b + 1, :], wsum[:])

    # audio_len = S - sum(mask)
    pmsum_col = sm.tile([16, 1], F32, tag="pmsum_col")
    V.tensor_reduce(pmsum_col[:], pm_f[:], mybir.AxisListType.X, OP.add)
    ps2 = ps_sm.tile([128, 128], F32, tag="pssm")
    P.matmul(ps2[0:1, 0:1], pmsum_col[:], cst["ones16"][:], start=True, stop=True)
    audio = sm.tile([1, 1], F32, tag="audio")
    V.tensor_scalar(audio[:], ps2[0:1, 0:1], -1.0, float(S), OP.mult, OP.add)

    # ---- scale = (tl + 1e-4) / w_sum ----
    tlf = sm.tile([1, 1], F32, tag="tlf")
    V.tensor_copy(tlf[:], tli[:])
    desired = sm.tile([1, 1], F32, tag="desired")
    V.tensor_scalar(desired[:], tlf[:], 1e-4, None, OP.add)
    r = sm.tile([1, 1], F32, tag="r")
    V.reciprocal(r[:], wsum[:])
    for _ in range(2):
        e = sm.tile([1, 1], F32, tag="e")
        V.tensor_tensor(e[:], wsum[:], r[:], OP.mult)
        e2 = sm.tile([1, 1], F32, tag="e2")
        V.tensor_scalar(e2[:], e[:], -1.0, 2.0, OP.mult, OP.add)
        r2 = sm.tile([1, 1], F32, tag="r")
        V.tensor_tensor(r2[:], r[:], e2[:], OP.mult)
        r = r2
    scale = sm.tile([1, 1], F32, tag="scale")
    V.tensor_tensor(scale[:], desired[:], r[:], OP.mult)
    ps_bc = ps_sm.tile([128, 128], F32, tag="pssm")
    P.matmul(ps_bc[0:16, 0:1], cst["ones_1x16"][:], scale[:], start=True, stop=True)
    scale16 = sm.tile([16, 1], F32, tag="scale16")
    V.tensor_copy(scale16[:], ps_bc[0:16, 0:1])

    # ---- wp = w*scale ; two-level scan -> h, hprev ----
    wp = sm.tile([16, 128], F32, tag="wp")
    V.tensor_scalar(wp[:], w[:], scale16[:], None, OP.mult)
    intra = sm.tile([16, 128], F32, tag="intra")
    V.tensor_tensor_scan(intra[:], wp[:], cst["z16"][:], 0.0, OP.add, OP.add)
    ps3 = ps_sm.tile([128, 128], F32, tag="pssm")
    P.matmul(ps3[0:16, 0:1], cst["triu"][:], intra[:, 127:128], start=True, stop=True)
    offs = sm.tile([16, 1], F32, tag="offs")
    V.tensor_copy(offs[:], ps3[0:16, 0:1])
    h = sm.tile([16, 128], F32, tag="h")
    V.tensor_scalar(h[:], intra[:], offs[:], None, OP.add)
    hprev = sm.tile([16, 128], F32, tag="hprev")
    V.tensor_copy(hprev[:, 1:128], h[:, 0:127])
    V.tensor_copy(hprev[:, 0:1], offs[:])

    # ---- rne / floors / fire / right_w / left_w ----
    rh = sm.tile([16, 128], F32, tag="rh")
    _rne(V, rh[:], h[:])
    gt = sm.tile([16, 128], F32, tag="gt")
    V.tensor_tensor(gt[:], rh[:], h[:], OP.is_gt)
    fh = sm.tile([16, 128], F32, tag="fh")
    V.tensor_tensor(fh[:], rh[:], gt[:], OP.subtract)
    rhp = sm.tile([16, 128], F32, tag="rhp")
    _rne(G, rhp[:], hprev[:])
    gtp = sm.tile([16, 128], F32, tag="gtp")
    V.tensor_tensor(gtp[:], rhp[:], hprev[:], OP.is_gt)
    fhp = sm.tile([16, 128], F32, tag="fhp")
    G.tensor_tensor(fhp[:], rhp[:], gtp[:], OP.subtract)
    fire = sm.tile([16, 128], F32, tag="fire")
    V.tensor_tensor(fire[:], fh[:], fhp[:], OP.is_gt)
    frac = sm.tile([16, 128], F32, tag="frac")
    V.tensor_tensor(frac[:], h[:], fh[:], OP.subtract)
    rw = sm.tile([16, 128], F32, tag="rw")
    V.tensor_tensor(rw[:], frac[:], fire[:], OP.mult)
    lw = sm.tile([16, 128], F32, tag="lw")
    V.tensor_tensor(lw[:], wp[:], rw[:], OP.subtract)
    nc.sync.dma_start(io["rw"][b].rearrange("(n c) -> n c", n=16), rw[:])
    nc.sync.dma_start(io["lw"][b].rearrange("(n c) -> n c", n=16), lw[:])

    # rh05 = rh + 0.5 (exact: integer + 0.5)
    rh05 = sm.tile([16, 128], F32, tag="rh05")
    V.tensor_scalar(rh05[:], rh[:], 0.5, None, OP.add)

    # ---- stack + transpose -> per-chunk columns: h, hprev, rh05, wp ----
    stack = sm.tile([128, 128], F32, tag="stack")
    G.memset(stack[:], 0.0)
    V.tensor_copy(stack[0:16, :], h[:])
    V.tensor_copy(stack[32:48, :], hprev[:])
    V.tensor_copy(stack[64:80, :], rh05[:])
    V.tensor_copy(stack[96:112, :], wp[:])
    ps_tr = ps_sm.tile([128, 128], F32, tag="pssm")
    P.matmul(ps_tr[:], stack[:], cst["ident"][:], is_transpose=True, start=True, stop=True)
    cols = sm.tile([128, 128], F32, tag="cols")
    V.tensor_copy(cols[:], ps_tr[:])
    # cols[:, 0+j]=h_col, 32+j=hprev_col, 64+j=rh05_col, 96+j=wp_col

    ps_out = pools["ps_out"].tile([128, 2048], F32, tag="ps_out")
    ps_dly = pools["ps_dly"].tile([1, 512], F32, tag="ps_dly")
    ps_sh = pools["ps_sh"].tile([1, 768], F32, tag="ps_sh")

    x_pool = pools["x"]
    trow = cst["trow"]
    zl = cst["zeros128"]

    # statically-known first matmul into each PSUM out-bank gets start=True
    first_bank_writer = {}
    for j in range(NCH):
        for kb in range(cfg["wwidth"] // 128):
            first_bank_writer.setdefault(wbase[j] // 128 + kb, j)

    xts = {}
    for j in range(NCH):
        if j % 2 == 0:
            # one 512KB DMA covers two chunks: partition p, free-block c = row 128c+p
            x2 = x_pool.tile([128, 2 * C], F32, tag="xt")
            nc.sync.dma_start(
                x2[:].rearrange("p (c d) -> p c d", c=2),
                io["x"][b, j * 128 : (j + 2) * 128, :].rearrange("(c p) d -> p c d", p=128),
            )
            xts[j] = x2[:, 0:C]
            xts[j + 1] = x2[:, C : 2 * C]
        xt = xts[j]
        if j == 0:
            # delay-row init; sigma-h init doubles as its constant step term
            P.matmul(ps_dly[0:1, 0:512], zl[:, 0:1], xt[:], start=True, stop=False,
                     skip_group_check=True)
            P.matmul(ps_sh[0:1, 0:512], cst["c128_16"][:], cst["step16"][:, 0:512],
                     start=True, stop=False, skip_group_check=True)
            P.matmul(ps_sh[0:1, 512:768], cst["c128_16"][:], cst["step16"][:, 512:768],
                     start=True, stop=False, skip_group_check=True)

        Wj, Gj = wbase[j], gbase[j]
        # banded operator A over t in [Wj, Wj+wwidth)
        d1 = pools["d1"].tile([128, wwidth], F32, tag="d1")
        A_.activation(d1[:], trow[:, Wj : Wj + wwidth], ACTF.Identity,
                      bias=cols[:, 0 + j : 1 + j], scale=-1.0)
        d2 = pools["d2"].tile([128, wwidth], F32, tag="d2")
        G.tensor_scalar(d2[:], trow[:, Wj + 1 : Wj + 1 + wwidth],
                        cols[:, 32 + j : 33 + j], None, OP.subtract)
        m = pools["m"].tile([128, wwidth], F32, tag="m")
        V.tensor_tensor(m[:], d1[:], d2[:], OP.min)
        At = pools["A"].tile([128, wwidth], F32, tag="At")
        G.tensor_scalar(At[:], m[:], cols[:, 96 + j : 97 + j], 0.0, OP.min, OP.max)
        # sigma-h comparison tile: M[s, jw] = 1[(Gj+jw) > rh_s + 0.5]
        Mt = pools["M"].tile([128, gwidth], F32, tag="Mt")
        V.tensor_scalar(Mt[:], trow[:, Gj : Gj + gwidth], cols[:, 64 + j : 65 + j],
                        None, OP.is_gt)
        # delay rhs column: s+1
        src_col = sm.tile([128, 1], F32, tag="src_col")
        G.tensor_scalar(src_col[:], cst["pcol"][:], float(j * 128 + 1), None, OP.add)

        for kb in range(wwidth // 128):
            bank = Wj // 128 + kb
            P.matmul(ps_out[:, bank * 512 : (bank + 1) * 512],
                     At[:, kb * 128 : (kb + 1) * 128], xt[:],
                     start=(first_bank_writer[bank] == j), stop=False,
                     skip_group_check=True)
        P.matmul(ps_dly[0:1, Wj : Wj + wwidth], src_col[:], At[:],
                 start=False, stop=False, skip_group_check=True)
        # sigma-h count matmuls; split only where the window would cross a PSUM bank
        gsplits = []
        gpos = 0
        while gpos < gwidth:
            seg = min(gwidth - gpos, 512 - ((Gj + gpos) % 512))
            gsplits.append((gpos, seg))
            gpos += seg
        for gs, seg in gsplits:
            P.matmul(ps_sh[0:1, Gj + gs : Gj + gs + seg],
                     cst["ones128"][:], Mt[:, gs : gs + seg],
                     start=False, stop=False, skip_group_check=True)

    # ---- flush output ----
    for kb in range(4):
        ob = pools["stage"].tile([128, C], F32, tag="ob")
        if kb < 3:
            A_.copy(ob[:], ps_out[:, kb * 512 : (kb + 1) * 512])
        else:
            V.tensor_copy(ob[:], ps_out[:, kb * 512 : (kb + 1) * 512])
        nc.sync.dma_start(io["out"][b, kb * 128 : (kb + 1) * 128, :], ob[:])

    # ---- flush delay ----
    d_sb = sm.tile([1, 512], F32, tag="d_sb")
    V.tensor_copy(d_sb[:], ps_dly[:])
    nc.sync.dma_start(io["dly"][b : b + 1, :], d_sb[:])

    # ---- durations ----
    sh_sb = sm.tile([1, 516], F32, tag="sh_sb")
    V.tensor_copy(sh_sb[:], ps_sh[0:1, 0:516])
    durA = sm.tile([1, 512], F32, tag="durA")
    V.tensor_tensor(durA[:], sh_sb[0:1, 1:513], sh_sb[0:1, 0:512], OP.subtract)
    durB = sm.tile([1, 512], F32, tag="durB")
    V.tensor_tensor(durB[:], sh_sb[0:1, 2:514], sh_sb[0:1, 1:513], OP.subtract)
    dd = sm.tile([1, 512], F32, tag="dd")
    V.tensor_tensor(dd[:], durB[:], durA[:], OP.subtract)
    dur_f = sm.tile([1, 512], F32, tag="dur_f")
    V.scalar_tensor_tensor(dur_f[:], dd[:], rh[0:1, 0:1], durA[:], OP.mult, OP.add)
    tdiff = sm.tile([1, 1], F32, tag="tdiff")
    V.tensor_tensor(tdiff[:], sh_sb[0:1, 512:513], sh_sb[0:1, 511:512], OP.subtract)
    sel = sm.tile([1, 1], F32, tag="sel")
    V.scalar_tensor_tensor(sel[:], tdiff[:], rh[0:1, 0:1], sh_sb[0:1, 511:512],
                           OP.mult, OP.add)
    tailv = sm.tile([1, 1], F32, tag="tailv")
    V.tensor_tensor(tailv[:], audio[:], sel[:], OP.subtract)
    V.tensor_copy(dur_f[0:1, 511:512], tailv[:])
    dur_i = sm.tile([1, 512], I32, tag="dur_i")
    V.tensor_copy(dur_i[:], dur_f[:])
    nc.sync.dma_start(io["dur"][b : b + 1, :], dur_i[:])


def build_program(cfg_name="banded"):
    cfg = CONFIGS[cfg_name]
    nc = bacc.Bacc("TRN2", target_bir_lowering=False, debug=False)
    io = {
        "x": nc.dram_tensor("x", [BLOC, S, C], F32, kind="ExternalInput").ap(),
        "iw": nc.dram_tensor("iw", [BLOC, S], F32, kind="ExternalInput").ap(),
        "pm": nc.dram_tensor("pm", [BLOC, S], U8, kind="ExternalInput").ap(),
        "tl": nc.dram_tensor("tl", [BLOC, 1], I32, kind="ExternalInput").ap(),
        "out": nc.dram_tensor("out", [BLOC, TMAX, C], F32, kind="ExternalOutput").ap(),
        "dly": nc.dram_tensor("dly", [BLOC, TMAX], F32, kind="ExternalOutput").ap(),
        "dur": nc.dram_tensor("dur", [BLOC, TMAX], I32, kind="ExternalOutput").ap(),
        "ws": nc.dram_tensor("ws", [BLOC, 1], F32, kind="ExternalOutput").ap(),
        "rw": nc.dram_tensor("rw", [BLOC, S], F32, kind="ExternalOutput").ap(),
        "lw": nc.dram_tensor("lw", [BLOC, S], F32, kind="ExternalOutput").ap(),
    }
    with tile.TileContext(nc) as tc:
        from contextlib import ExitStack

        with ExitStack() as ctx:
            pools = {
                "const": ctx.enter_context(tc.tile_pool(name="const", bufs=1)),
                "sm": ctx.enter_context(tc.tile_pool(name="sm", bufs=2)),
                "x": ctx.enter_context(tc.tile_pool(name="x", bufs=4)),
                "d1": ctx.enter_context(tc.tile_pool(name="d1", bufs=3)),
                "d2": ctx.enter_context(tc.tile_pool(name="d2", bufs=3)),
                "m": ctx.enter_context(tc.tile_pool(name="m", bufs=3)),
                "A": ctx.enter_context(tc.tile_pool(name="A", bufs=3)),
                "M": ctx.enter_context(tc.tile_pool(name="M", bufs=3)),
                "stage": ctx.enter_context(tc.tile_pool(name="stage", bufs=3)),
                "ps_out": ctx.enter_context(tc.tile_pool(name="ps_out", bufs=1, space="PSUM")),
                "ps_dly": ctx.enter_context(tc.tile_pool(name="ps_dly", bufs=1, space="PSUM")),
                "ps_sh": ctx.enter_context(tc.tile_pool(name="ps_sh", bufs=1, space="PSUM")),
                "ps_sm": ctx.enter_context(tc.tile_pool(name="ps_sm", bufs=1, space="PSUM")),
            }
            cst = _emit_consts(nc, tc, pools, cfg)
            for b in range(BLOC):
                _emit_sample(nc, tc, pools, cst, io, b, cfg)
    nc.finalize()
    return nc


_NC_CACHE = {}


def _get_nc(cfg_name="banded"):
    if cfg_name not in _NC_CACHE:
        _NC_CACHE[cfg_name] = build_program(cfg_name)
    return _NC_CACHE[cfg_name]


def _pick_config(iw, pm, tl):
    """Verify the static banded windows cover the data (margin >= 4); else 'full'.

    Host-side validation only — the chosen program is correct for any data that
    fits its windows, and 'full' is correct unconditionally.
    """
    w = np.clip(iw.astype(np.float64), 0.0, 1.0) * (1.0 - pm.astype(np.float64))
    ws = w.sum(axis=1)
    scale = (tl.astype(np.float64).reshape(-1) + 1e-4) / ws
    wpd = w * scale[:, None]
    cs = np.cumsum(wpd, axis=1).reshape(-1, NCH, CH)
    hi = cs[:, :, -1]  # chunk-end h
    lo = hi - np.diff(np.concatenate([np.zeros((len(ws), 1)), hi], axis=1))  # chunk starts
    M = 4.0
    for c in range(NCH):
        wlo, whi = WBASE[c], WBASE[c] + 256
        if wlo > 0 and (lo[:, c] < wlo + M).any():
            return "full"
        if whi < TMAX and (hi[:, c] > whi - M).any():
            return "full"
        glo, ghi = GBASE[c], GBASE[c] + 256
        if glo > 0 and (lo[:, c] + 0.5 < glo + M).any():
            return "full"
        if ghi < 514 and (hi[:, c] + 0.5 > ghi - M).any():
            return "full"
    return "banded"


def kernel(inputs, input_weights, padding_mask, target_lengths, Tmax):
    x = np.ascontiguousarray(np.asarray(inputs, dtype=np.float32))
    iw = np.ascontiguousarray(np.asarray(input_weights, dtype=np.float32))
    pm = np.ascontiguousarray(np.asarray(padding_mask).astype(np.uint8))
    tl = np.ascontiguousarray(np.asarray(target_lengths, dtype=np.int32)).reshape(B, 1)
    assert int(Tmax) == TMAX and x.shape == (B, S, C)

    cfg_name = _pick_config(iw, pm, tl)
    nc = _get_nc(cfg_name)
    in_maps = []
    for i in range(NCORES):
        sl = slice(i * BLOC, (i + 1) * BLOC)
        in_maps.append({"x": x[sl], "iw": iw[sl], "pm": pm[sl], "tl": tl[sl]})
    res = run_bass_kernel_spmd(nc, in_maps, core_ids=list(range(NCORES)))
    rs = res.results
    output = np.concatenate([r["out"] for r in rs], axis=0)
    delay = np.concatenate([r["dly"] for r in rs], axis=0)
    dur = np.concatenate([r["dur"] for r in rs], axis=0).astype(np.int32)
    w_sum = np.concatenate([r["ws"] for r in rs], axis=0).reshape(B)
    right_w = np.concatenate([r["rw"] for r in rs], axis=0)
    left_w = np.concatenate([r["lw"] for r in rs], axis=0)
    return output, delay, dur, w_sum, right_w, left_w
